# revision 8
# baseline (speedup 1.0000x reference)
"""GroupHadamardLayer (segment_reduce) Trainium2 kernel — PE matvec version.

The reference is linear in x, so it collapses to out = x @ w with
    w[group_idx[n, g]] += gc_w[n, g] * diag_w[n] * fc_w[n, 0]
(scatter-add — exact for duplicate indices too).

Device kernel: memory-bound matvec done on the TensorEngine. x is sharded
by batch across 8 cores (2048 rows each). The host transposes each shard
to xT [F=2048 feats, R=2048 rows]. Most feature tiles are quantized
per-row to int8 (x_q = round(x / d_r), d_r = max|x_r|/127; the scale is
folded back on the host as out *= d_r) and upcast to bf16 on-chip by the
DVE (tensor_copy, 2x_2p). The first and last few tiles are sent as bf16
directly (2x the bytes but no cast), which takes the cast off the
critical path at the pipeline fill and drain.

All x DMAs are issued up-front, split across the two HWDGE rings
(Sync + Scalar) with balanced bytes — each ring is FIFO and serializes
the ~1us completion receipt per DMA. Per 128-feature tile, 4 PE matmuls
(stationary = w-slice [128, 1] bf16, moving [128, 512]) accumulate the
16 feature tiles into 4 PSUM banks. A short burst of dummy matmuls at
kernel start warms the PE HAM clock gate (the PE runs at half clock
until it has been busy for a few us). PSUM [1, 512] x4 -> SBUF -> two
4 KiB DMAs out (one per ring). Host folds the per-row scales back in.
Event-semaphore count is kept low (coarse casts, no buffer recycling)
because the kernel postamble walks every event sem used (~130 ns each).
"""

import os
import sys
from contextlib import ExitStack

sys.path.insert(0, "/opt/trn_rl_repo")

import ml_dtypes
import numpy as np

from concourse import bacc, bass, tile
from concourse.bass_utils import run_bass_kernel_spmd

mybir = bass.mybir
F32 = mybir.dt.float32
BF16 = mybir.dt.bfloat16
I8 = mybir.dt.int8

B, F = 16384, 2048
N_CORES = 8
ROWS = B // N_CORES  # 2048 rows per core
P = 128
N_FT = F // P  # 16 feature tiles
RC = 512  # rows per PSUM bank (512 f32 = one bank)
N_RC = ROWS // RC  # 4

N_WARMUP = int(os.environ.get("KWARMUP", "20"))

# Chunk schedule: (f-tiles, dtype, ring). Rings byte-balanced; bf16 tiles
# need no cast (pipeline head + tail), int8 tiles are cast on DVE.
# fts are consumed by the PE in order f0..f15.
CHUNKS = [
    ((0,), "bf16", "sync"),
    ((1, 2), "int8", "scalar"),
    ((3, 4), "int8", "sync"),
    ((5, 6), "int8", "scalar"),
    ((7, 8), "int8", "sync"),
    ((9, 10, 11), "int8", "scalar"),
    ((12, 13), "bf16", "sync"),
    ((14, 15), "bf16", "scalar"),
]
BF16_FTS = sorted(ft for fts, dt, _ in CHUNKS if dt == "bf16" for ft in fts)
I8_FTS = sorted(ft for fts, dt, _ in CHUNKS if dt == "int8" for ft in fts)

_NC = None
LAST_RESULT = None  # BassKernelResults of the most recent run (for test.py)


def _build_nc():
    nc = bacc.Bacc("TRN2", target_bir_lowering=False, debug=False)
    # int8 f-tiles and bf16 f-tiles live in two dram tensors, each packed
    # in f-tile order: [n_tiles*128 feats, ROWS].
    xt8 = nc.dram_tensor("xt8", [len(I8_FTS) * P, ROWS], I8, kind="ExternalInput")
    xt16 = nc.dram_tensor(
        "xt16", [len(BF16_FTS) * P, ROWS], BF16, kind="ExternalInput"
    )
    wst = nc.dram_tensor("wst", [P, N_FT], BF16, kind="ExternalInput")
    out = nc.dram_tensor("out", [1, ROWS], F32, kind="ExternalOutput")
    i8_pos = {ft: i for i, ft in enumerate(I8_FTS)}
    bf_pos = {ft: i for i, ft in enumerate(BF16_FTS)}

    with tile.TileContext(nc) as tc:
        with (
            tc.tile_pool(name="xi", bufs=1) as xi,
            tc.tile_pool(name="xb", bufs=1) as xb,
            tc.tile_pool(name="wp", bufs=1) as wp,
            tc.psum_pool(name="pp", bufs=1) as pp,
        ):
            # PE HAM warmup: garbage matmuls (no data deps) keep the PE busy
            # from t=0 so the clock gate is open when real tiles arrive.
            warm_t = wp.tile([P, 256], BF16)
            psums = [
                pp.tile([1, RC], F32, name=f"psum{rc}") for rc in range(N_RC)
            ]
            if N_WARMUP:
                warm_ps = pp.tile([1, 128], F32)
                nc.gpsimd.memset(warm_t[:], 0)
                for _ in range(N_WARMUP):
                    nc.tensor.matmul(
                        warm_ps[:, :], lhsT=warm_t[:, 0:1], rhs=warm_t[:, 0:256:2],
                        start=True, stop=True,
                    )

            w_t = wp.tile([P, N_FT], BF16)
            out_t = wp.tile([1, ROWS], F32)

            # Issue every DMA up-front; rings run them back-to-back.
            ft_tile = {}  # ft -> (sbuf tile ap provider, needs_cast)
            w_issued = False
            for ci, (fts, dt, ring) in enumerate(CHUNKS):
                size = len(fts)
                eng = nc.sync if ring == "sync" else nc.scalar
                if dt == "int8":
                    t0 = i8_pos[fts[0]]
                    tl = xi.tile([P, size, ROWS], I8, name=f"xi{ci}")
                    src = xt8.ap()[t0 * P : (t0 + size) * P, :]
                else:
                    t0 = bf_pos[fts[0]]
                    tl = xb.tile([P, size, ROWS], BF16, name=f"xb{ci}")
                    src = xt16.ap()[t0 * P : (t0 + size) * P, :]
                eng.dma_start(tl[:], src.rearrange("(g p) r -> p g r", p=P))
                for g, ft in enumerate(fts):
                    ft_tile[ft] = (tl, g, dt == "int8")
                if not w_issued:
                    # w rides second on the other ring (tiny, needed by f0).
                    other = nc.scalar if ring == "sync" else nc.sync
                    other.dma_start(w_t[:], wst.ap())
                    w_issued = True

            # Casts (DVE only; ACT's activation-copy is ~1.7x slower and its
            # queue of DMA issues would delay critical casts).
            cast_tile = {}
            for ci, (fts, dt, ring) in enumerate(CHUNKS):
                if dt != "int8":
                    continue
                tl, _, _ = ft_tile[fts[0]]
                xc = xb.tile([P, len(fts), ROWS], BF16, name=f"xc{ci}")
                nc.vector.tensor_copy(out=xc[:], in_=tl[:])
                for g, ft in enumerate(fts):
                    cast_tile[ft] = (xc, g)

            for ft in range(N_FT):
                if ft in cast_tile:
                    tl, g = cast_tile[ft]
                else:
                    tl, g, _ = ft_tile[ft]
                for rc in range(N_RC):
                    nc.tensor.matmul(
                        psums[rc][:, :],
                        lhsT=w_t[:, ft : ft + 1],
                        rhs=tl[:, g, rc * RC : (rc + 1) * RC],
                        start=(ft == 0),
                        stop=(ft == N_FT - 1),
                    )

            # Per-bank evacuation; each copy only waits for its own bank's
            # last matmul. Two half-outputs, one per ring.
            for rc in range(N_RC):
                dst = out_t[:, rc * RC : (rc + 1) * RC]
                if rc % 2 == 0:
                    nc.scalar.copy(out=dst, in_=psums[rc][:, :])
                else:
                    nc.vector.tensor_copy(out=dst, in_=psums[rc][:, :])
            half = ROWS // 2
            nc.scalar.dma_start(out.ap()[:, :half], out_t[:, :half])
            nc.sync.dma_start(out.ap()[:, half:], out_t[:, half:])
    nc.finalize()
    return nc


def kernel(x, group_idx, gc_w, diag_w, fc_w):
    global _NC, LAST_RESULT
    x = np.ascontiguousarray(np.asarray(x, dtype=np.float32))
    gi = np.asarray(group_idx).astype(np.int64)
    gc_w = np.asarray(gc_w, dtype=np.float32)
    diag_w = np.asarray(diag_w, dtype=np.float32).reshape(-1)
    fc_w = np.asarray(fc_w, dtype=np.float32).reshape(-1, 1)

    # Fold everything linear into one combined weight vector (exact).
    coef = gc_w * diag_w[:, None] * fc_w  # [256, 8]
    w = np.zeros(F, dtype=np.float32)
    np.add.at(w, gi.ravel(), coef.ravel().astype(np.float32))
    # stationary layout: wst[p, t] = w[t*128 + p]
    wst = np.ascontiguousarray(w.reshape(N_FT, P).T).astype(ml_dtypes.bfloat16)

    # Per-row int8 scales; bf16 tiles are sent pre-scaled by 1/d_r too so a
    # single host-side out *= d_r fixes everything.
    d = np.maximum(np.abs(x).max(axis=1), 1e-30) / 127.0  # [B]
    xs = x / d[:, None]
    xq = np.rint(xs).astype(np.int8)
    xb16 = xs.astype(ml_dtypes.bfloat16)

    i8_rows = np.array([list(range(ft * P, (ft + 1) * P)) for ft in I8_FTS]
                       ).reshape(-1)
    bf_rows = np.array([list(range(ft * P, (ft + 1) * P)) for ft in BF16_FTS]
                       ).reshape(-1)
    in_maps = []
    for i in range(N_CORES):
        sl = slice(i * ROWS, (i + 1) * ROWS)
        xqT = np.ascontiguousarray(xq[sl].T[i8_rows])
        xbT = np.ascontiguousarray(xb16[sl].T[bf_rows])
        in_maps.append({"xt8": xqT, "xt16": xbT, "wst": wst})

    if _NC is None:
        _NC = _build_nc()

    trace = bool(int(os.environ.get("TRN_KERNEL_TRACE", "0")))
    LAST_RESULT = run_bass_kernel_spmd(
        _NC, in_maps, list(range(N_CORES)), trace=trace
    )
    outs = [
        LAST_RESULT.results[i]["out"].reshape(ROWS).astype(np.float32)
        for i in range(N_CORES)
    ]
    full = np.concatenate(outs) * d
    return full.reshape(B, 1).astype(np.float32)


# revision 9
# speedup vs baseline: 1.0492x; 1.0492x over previous
"""GroupHadamardLayer (segment_reduce) Trainium2 kernel — PE matvec version.

The reference is linear in x, so it collapses to out = x @ w with
    w[group_idx[n, g]] += gc_w[n, g] * diag_w[n] * fc_w[n, 0]
(scatter-add — exact for duplicate indices too).

Device kernel: memory-bound matvec on the TensorEngine. x is sharded by
batch across 8 cores (2048 rows each). The host transposes each shard to
xT [F=2048 feats, R=2048 rows]. Middle feature tiles are quantized
per-row to int8 (x_q = round(x / d_r), d_r = max|x_r|/127; the scale is
folded back on the host as out *= d_r) and upcast to bf16 on-chip by the
DVE (tensor_copy, 2x_2p mode, ~1.1us per tile). Head/tail tiles are sent
as bf16 directly (2x bytes, no cast) so no cast sits on the pipeline
fill or drain critical path. DMAs alternate between the two HWDGE rings
(Sync + Scalar): each ring is FIFO and serializes its ~1us per-DMA
completion receipt. Issue order is just-in-time (chunk i+1's DMA issues
while chunk i computes) — issuing everything up-front makes early
chunks' completion sems fire late because all queued DMAs fair-share the
16 SDMA engines.

Per 128-feature tile, 4 PE matmuls (stationary = w-slice [128, 1] bf16,
moving [128, 512]) accumulate 16 feature tiles into 4 PSUM banks. A
burst of dummy matmuls at kernel start warms the PE HAM clock gate (the
PE runs at 1.2 GHz until it has been busy ~3.4us sustained; 2.4 GHz
after). PSUM [1, 512] x4 -> SBUF -> two 4 KiB DMAs out (one per ring).
Host folds the per-row scales back in.
"""

import os
import sys
from contextlib import ExitStack

sys.path.insert(0, "/opt/trn_rl_repo")

import ml_dtypes
import numpy as np

from concourse import bacc, bass, tile
from concourse.bass_utils import run_bass_kernel_spmd

mybir = bass.mybir
F32 = mybir.dt.float32
BF16 = mybir.dt.bfloat16
I8 = mybir.dt.int8

B, F = 16384, 2048
N_CORES = 8
ROWS = B // N_CORES  # 2048 rows per core
P = 128
N_FT = F // P  # 16 feature tiles
RC = 512  # rows per PSUM bank (512 f32 = one bank)
N_RC = ROWS // RC  # 4

N_WARMUP = int(os.environ.get("KWARMUP", "48"))

# Chunk schedule: (f-tiles, dtype, ring), consumed by the PE in order.
CHUNKS = [
    ((0,), "bf16", "sync"),
    ((1, 2), "int8", "scalar"),
    ((3, 4), "int8", "sync"),
    ((5, 6), "int8", "scalar"),
    ((7, 8), "int8", "sync"),
    ((9, 10), "int8", "scalar"),
    ((11, 12), "int8", "sync"),
    ((13,), "int8", "scalar"),
    ((14,), "bf16", "sync"),
    ((15,), "bf16", "scalar"),
]
BF16_FTS = sorted(ft for fts, dt, _ in CHUNKS if dt == "bf16" for ft in fts)
I8_FTS = sorted(ft for fts, dt, _ in CHUNKS if dt == "int8" for ft in fts)

_NC = None
LAST_RESULT = None  # BassKernelResults of the most recent run (for test.py)


def _build_nc():
    nc = bacc.Bacc("TRN2", target_bir_lowering=False, debug=False)
    xt8 = nc.dram_tensor("xt8", [len(I8_FTS) * P, ROWS], I8, kind="ExternalInput")
    xt16 = nc.dram_tensor(
        "xt16", [len(BF16_FTS) * P, ROWS], BF16, kind="ExternalInput"
    )
    wst = nc.dram_tensor("wst", [P, N_FT], BF16, kind="ExternalInput")
    out = nc.dram_tensor("out", [1, ROWS], F32, kind="ExternalOutput")
    i8_pos = {ft: i for i, ft in enumerate(I8_FTS)}
    bf_pos = {ft: i for i, ft in enumerate(BF16_FTS)}

    with tile.TileContext(nc) as tc:
        with (
            tc.tile_pool(name="xi", bufs=1) as xi,
            tc.tile_pool(name="xb", bufs=1) as xb,
            tc.tile_pool(name="wp", bufs=1) as wp,
            tc.psum_pool(name="pp", bufs=1) as pp,
        ):
            # PE HAM warmup: garbage matmuls (no data deps) keep the PE busy
            # from t=0 so the clock gate is open when real tiles arrive.
            warm_t = wp.tile([P, P], BF16)
            psums = [
                pp.tile([1, RC], F32, name=f"psum{rc}") for rc in range(N_RC)
            ]
            if N_WARMUP:
                warm_ps = pp.tile([1, P], F32)
                nc.gpsimd.memset(warm_t[:], 0)
                for _ in range(N_WARMUP):
                    nc.tensor.matmul(
                        warm_ps[:, :], lhsT=warm_t[:, 0:1], rhs=warm_t[:],
                        start=True, stop=True,
                    )

            w_t = wp.tile([P, N_FT], BF16)
            nc.scalar.dma_start(w_t[:], wst.ap())
            out_t = wp.tile([1, ROWS], F32)

            for ci, (fts, dt, ring) in enumerate(CHUNKS):
                size = len(fts)
                eng = nc.sync if ring == "sync" else nc.scalar
                if dt == "int8":
                    t0 = i8_pos[fts[0]]
                    tl = xi.tile([P, size, ROWS], I8, name=f"xi{ci}")
                    src = xt8.ap()[t0 * P : (t0 + size) * P, :]
                else:
                    t0 = bf_pos[fts[0]]
                    tl = xb.tile([P, size, ROWS], BF16, name=f"xb{ci}")
                    src = xt16.ap()[t0 * P : (t0 + size) * P, :]
                eng.dma_start(tl[:], src.rearrange("(g p) r -> p g r", p=P))
                if dt == "int8":
                    xc = xb.tile([P, size, ROWS], BF16, name=f"xc{ci}")
                    nc.vector.tensor_copy(out=xc[:], in_=tl[:])
                    tl = xc
                for g, ft in enumerate(fts):
                    for rc in range(N_RC):
                        nc.tensor.matmul(
                            psums[rc][:, :],
                            lhsT=w_t[:, ft : ft + 1],
                            rhs=tl[:, g, rc * RC : (rc + 1) * RC],
                            start=(ft == 0),
                            stop=(ft == N_FT - 1),
                        )

            # Per-bank evacuation; each copy only waits for its own bank's
            # last matmul. Two half-outputs, one per ring.
            for rc in range(N_RC):
                dst = out_t[:, rc * RC : (rc + 1) * RC]
                if rc % 2 == 0:
                    nc.scalar.copy(out=dst, in_=psums[rc][:, :])
                else:
                    nc.vector.tensor_copy(out=dst, in_=psums[rc][:, :])
            half = ROWS // 2
            nc.scalar.dma_start(out.ap()[:, :half], out_t[:, :half])
            nc.sync.dma_start(out.ap()[:, half:], out_t[:, half:])
    nc.finalize()
    return nc


def kernel(x, group_idx, gc_w, diag_w, fc_w):
    global _NC, LAST_RESULT
    x = np.ascontiguousarray(np.asarray(x, dtype=np.float32))
    gi = np.asarray(group_idx).astype(np.int64)
    gc_w = np.asarray(gc_w, dtype=np.float32)
    diag_w = np.asarray(diag_w, dtype=np.float32).reshape(-1)
    fc_w = np.asarray(fc_w, dtype=np.float32).reshape(-1, 1)

    # Fold everything linear into one combined weight vector (exact).
    coef = gc_w * diag_w[:, None] * fc_w  # [256, 8]
    w = np.zeros(F, dtype=np.float32)
    np.add.at(w, gi.ravel(), coef.ravel().astype(np.float32))
    # stationary layout: wst[p, t] = w[t*128 + p]
    wst = np.ascontiguousarray(w.reshape(N_FT, P).T).astype(ml_dtypes.bfloat16)

    # Per-row scales; bf16 tiles are sent pre-scaled by 1/d_r too, so one
    # host-side out *= d_r fixes everything.
    d = np.maximum(np.abs(x).max(axis=1), 1e-30) / 127.0  # [B]
    xs = x / d[:, None]
    xq = np.rint(xs).astype(np.int8)
    xb16 = xs.astype(ml_dtypes.bfloat16)

    i8_rows = np.array([list(range(ft * P, (ft + 1) * P)) for ft in I8_FTS]
                       ).reshape(-1)
    bf_rows = np.array([list(range(ft * P, (ft + 1) * P)) for ft in BF16_FTS]
                       ).reshape(-1)
    in_maps = []
    for i in range(N_CORES):
        sl = slice(i * ROWS, (i + 1) * ROWS)
        xqT = np.ascontiguousarray(xq[sl].T[i8_rows])
        xbT = np.ascontiguousarray(xb16[sl].T[bf_rows])
        in_maps.append({"xt8": xqT, "xt16": xbT, "wst": wst})

    if _NC is None:
        _NC = _build_nc()

    trace = bool(int(os.environ.get("TRN_KERNEL_TRACE", "0")))
    LAST_RESULT = run_bass_kernel_spmd(
        _NC, in_maps, list(range(N_CORES)), trace=trace
    )
    outs = [
        LAST_RESULT.results[i]["out"].reshape(ROWS).astype(np.float32)
        for i in range(N_CORES)
    ]
    full = np.concatenate(outs) * d
    return full.reshape(B, 1).astype(np.float32)


# revision 11
# speedup vs baseline: 1.0928x; 1.0416x over previous
"""GroupHadamardLayer (segment_reduce) Trainium2 kernel — PE matvec version.

The reference is linear in x, so it collapses to out = x @ w with
    w[group_idx[n, g]] += gc_w[n, g] * diag_w[n] * fc_w[n, 0]
(scatter-add — exact for duplicate indices too).

Device kernel: memory-bound matvec on the TensorEngine. x is sharded by
batch across 8 cores (2048 rows each). The host transposes each shard to
xT [F=2048 feats, R=2048 rows]. Middle feature tiles are quantized
per-row to int8 (x_q = round(x / d_r), d_r = max|x_r|/127; the scale is
folded back on the host as out *= d_r) and upcast to bf16 on-chip by the
DVE (tensor_copy, 2x_2p mode, ~1.1us per tile). Head/tail tiles are sent
as bf16 directly (2x bytes, no cast) so no cast sits on the pipeline
fill or drain critical path. DMAs alternate between the two HWDGE rings
(Sync + Scalar): each ring is FIFO and serializes its ~1us per-DMA
completion receipt. Issue order is just-in-time (chunk i+1's DMA issues
while chunk i computes) — issuing everything up-front makes early
chunks' completion sems fire late because all queued DMAs fair-share the
16 SDMA engines.

Per 128-feature tile, 4 PE matmuls (stationary = w-slice [128, 1] bf16,
moving [128, 512]) accumulate 16 feature tiles into 4 PSUM banks. A
burst of dummy matmuls at kernel start warms the PE HAM clock gate (the
PE runs at 1.2 GHz until it has been busy ~3.4us sustained; 2.4 GHz
after). PSUM [1, 512] x4 -> SBUF -> two 4 KiB DMAs out (one per ring).
Host folds the per-row scales back in.
"""

import os
import sys
from contextlib import ExitStack

sys.path.insert(0, "/opt/trn_rl_repo")

import ml_dtypes
import numpy as np

from concourse import bacc, bass, tile
from concourse.bass_utils import run_bass_kernel_spmd

mybir = bass.mybir
F32 = mybir.dt.float32
BF16 = mybir.dt.bfloat16
I8 = mybir.dt.int8

B, F = 16384, 2048
N_CORES = 8
ROWS = B // N_CORES  # 2048 rows per core
P = 128
N_FT = F // P  # 16 feature tiles
RC = 512  # rows per PSUM bank (512 f32 = one bank)
N_RC = ROWS // RC  # 4

N_WARMUP = int(os.environ.get("KWARMUP", "48"))

# Chunk schedule: (f-tiles, dtype, ring), consumed by the PE in order.
CHUNKS = [
    ((0,), "bf16", "sync"),
    ((1, 2), "int8", "scalar"),
    ((3, 4), "int8", "sync"),
    ((5, 6), "int8", "scalar"),
    ((7, 8), "int8", "sync"),
    ((9, 10), "int8", "scalar"),
    ((11, 12), "int8", "sync"),
    ((13,), "int8", "scalar"),
    ((14,), "bf16", "sync"),
    ((15,), "bf16", "scalar"),
]
BF16_FTS = sorted(ft for fts, dt, _ in CHUNKS if dt == "bf16" for ft in fts)
I8_FTS = sorted(ft for fts, dt, _ in CHUNKS if dt == "int8" for ft in fts)

_NC = None
LAST_RESULT = None  # BassKernelResults of the most recent run (for test.py)


def _build_nc():
    nc = bacc.Bacc("TRN2", target_bir_lowering=False, debug=False)
    xt8 = nc.dram_tensor("xt8", [len(I8_FTS) * P, ROWS], I8, kind="ExternalInput")
    xt16 = nc.dram_tensor(
        "xt16", [len(BF16_FTS) * P, ROWS], BF16, kind="ExternalInput"
    )
    wst = nc.dram_tensor("wst", [P, N_FT], BF16, kind="ExternalInput")
    out = nc.dram_tensor("out", [1, ROWS], F32, kind="ExternalOutput")
    i8_pos = {ft: i for i, ft in enumerate(I8_FTS)}
    bf_pos = {ft: i for i, ft in enumerate(BF16_FTS)}

    with tile.TileContext(nc) as tc:
        with (
            tc.tile_pool(name="xi", bufs=1) as xi,
            tc.tile_pool(name="xb", bufs=1) as xb,
            tc.tile_pool(name="wp", bufs=1) as wp,
            tc.psum_pool(name="pp", bufs=1) as pp,
        ):
            # PE HAM warmup: garbage matmuls (no data deps) keep the PE busy
            # from t=0 so the clock gate is open when real tiles arrive.
            warm_t = wp.tile([P, P], BF16)
            psums = [
                pp.tile([1, RC], F32, name=f"psum{rc}") for rc in range(N_RC)
            ]
            if N_WARMUP:
                warm_ps = pp.tile([1, P], F32)
                nc.gpsimd.memset(warm_t[:], 0)
                for _ in range(N_WARMUP):
                    nc.tensor.matmul(
                        warm_ps[:, :], lhsT=warm_t[:, 0:1], rhs=warm_t[:],
                        start=True, stop=True,
                    )

            w_t = wp.tile([P, N_FT], BF16)
            out_t = wp.tile([1, ROWS], F32)

            for ci, (fts, dt, ring) in enumerate(CHUNKS):
                size = len(fts)
                eng = nc.sync if ring == "sync" else nc.scalar
                if dt == "int8":
                    t0 = i8_pos[fts[0]]
                    tl = xi.tile([P, size, ROWS], I8, name=f"xi{ci}")
                    src = xt8.ap()[t0 * P : (t0 + size) * P, :]
                else:
                    t0 = bf_pos[fts[0]]
                    tl = xb.tile([P, size, ROWS], BF16, name=f"xb{ci}")
                    src = xt16.ap()[t0 * P : (t0 + size) * P, :]
                eng.dma_start(tl[:], src.rearrange("(g p) r -> p g r", p=P))
                if ci == 0:
                    # w rides second on the same ring as chunk 0 (tiny; done
                    # before f0's first matmul needs it). Keeping it off the
                    # other ring matters: each ring fully serializes
                    # [desc-gen -> transfer -> receipt] per DMA, so a leading
                    # w would delay that ring's first x chunk by ~2us.
                    nc.sync.dma_start(w_t[:], wst.ap())
                if dt == "int8":
                    xc = xb.tile([P, size, ROWS], BF16, name=f"xc{ci}")
                    nc.vector.tensor_copy(out=xc[:], in_=tl[:])
                    tl = xc
                for g, ft in enumerate(fts):
                    for rc in range(N_RC):
                        nc.tensor.matmul(
                            psums[rc][:, :],
                            lhsT=w_t[:, ft : ft + 1],
                            rhs=tl[:, g, rc * RC : (rc + 1) * RC],
                            start=(ft == 0),
                            stop=(ft == N_FT - 1),
                        )

            # Per-bank evacuation; each copy only waits for its own bank's
            # last matmul. Two half-outputs, one per ring.
            for rc in range(N_RC):
                dst = out_t[:, rc * RC : (rc + 1) * RC]
                if rc % 2 == 0:
                    nc.scalar.copy(out=dst, in_=psums[rc][:, :])
                else:
                    nc.vector.tensor_copy(out=dst, in_=psums[rc][:, :])
            half = ROWS // 2
            nc.scalar.dma_start(out.ap()[:, :half], out_t[:, :half])
            nc.sync.dma_start(out.ap()[:, half:], out_t[:, half:])
    nc.finalize()
    return nc


def kernel(x, group_idx, gc_w, diag_w, fc_w):
    global _NC, LAST_RESULT
    x = np.ascontiguousarray(np.asarray(x, dtype=np.float32))
    gi = np.asarray(group_idx).astype(np.int64)
    gc_w = np.asarray(gc_w, dtype=np.float32)
    diag_w = np.asarray(diag_w, dtype=np.float32).reshape(-1)
    fc_w = np.asarray(fc_w, dtype=np.float32).reshape(-1, 1)

    # Fold everything linear into one combined weight vector (exact).
    coef = gc_w * diag_w[:, None] * fc_w  # [256, 8]
    w = np.zeros(F, dtype=np.float32)
    np.add.at(w, gi.ravel(), coef.ravel().astype(np.float32))
    # stationary layout: wst[p, t] = w[t*128 + p]
    wst = np.ascontiguousarray(w.reshape(N_FT, P).T).astype(ml_dtypes.bfloat16)

    # Per-row scales; bf16 tiles are sent pre-scaled by 1/d_r too, so one
    # host-side out *= d_r fixes everything.
    d = np.maximum(np.abs(x).max(axis=1), 1e-30) / 127.0  # [B]
    xs = x / d[:, None]
    xq = np.rint(xs).astype(np.int8)
    xb16 = xs.astype(ml_dtypes.bfloat16)

    i8_rows = np.array([list(range(ft * P, (ft + 1) * P)) for ft in I8_FTS]
                       ).reshape(-1)
    bf_rows = np.array([list(range(ft * P, (ft + 1) * P)) for ft in BF16_FTS]
                       ).reshape(-1)
    in_maps = []
    for i in range(N_CORES):
        sl = slice(i * ROWS, (i + 1) * ROWS)
        xqT = np.ascontiguousarray(xq[sl].T[i8_rows])
        xbT = np.ascontiguousarray(xb16[sl].T[bf_rows])
        in_maps.append({"xt8": xqT, "xt16": xbT, "wst": wst})

    if _NC is None:
        _NC = _build_nc()

    trace = bool(int(os.environ.get("TRN_KERNEL_TRACE", "0")))
    LAST_RESULT = run_bass_kernel_spmd(
        _NC, in_maps, list(range(N_CORES)), trace=trace
    )
    outs = [
        LAST_RESULT.results[i]["out"].reshape(ROWS).astype(np.float32)
        for i in range(N_CORES)
    ]
    full = np.concatenate(outs) * d
    return full.reshape(B, 1).astype(np.float32)


# revision 12
# speedup vs baseline: 1.1218x; 1.0265x over previous
"""GroupHadamardLayer (segment_reduce) Trainium2 kernel — PE matvec version.

The reference is linear in x, so it collapses to out = x @ w with
    w[group_idx[n, g]] += gc_w[n, g] * diag_w[n] * fc_w[n, 0]
(scatter-add — exact for duplicate indices too).

Device kernel: memory-bound matvec on the TensorEngine. x is sharded by
batch across 8 cores (2048 rows each). The host transposes each shard to
xT [F=2048 feats, R=2048 rows]. Middle feature tiles are quantized
per-row to int8 (x_q = round(x / d_r), d_r = max|x_r|/127; the scale is
folded back on the host as out *= d_r) and upcast to bf16 on-chip by the
DVE (tensor_copy, 2x_2p mode, ~1.1us per tile). Head/tail tiles are sent
as bf16 directly (2x bytes, no cast): the head tile so no cast sits on
the pipeline-fill critical path, the tail tiles so the PE can consume
them the moment they land. The folded weight vector rides as 16 extra
bf16 columns appended to chunk 0's buffer (no separate DMA — each DMA
has ~3.5us of issue+completion latency). DMAs alternate between the two
HWDGE rings (Sync + Scalar). Per 128-feature tile, 4 PE matmuls
(stationary = w-slice [128, 1] bf16, moving [128, 512]) accumulate the
16 feature tiles into 4 PSUM banks. A burst of dummy matmuls at kernel
start warms the PE HAM clock gate (the PE runs at 1.2 GHz until it has
been busy ~3.4us sustained; 2.4 GHz after). PSUM [1, 512] x4 -> SBUF ->
two 4 KiB DMAs out (one per ring). Host folds the per-row scales back.
"""

import os
import sys
from contextlib import ExitStack

sys.path.insert(0, "/opt/trn_rl_repo")

import ml_dtypes
import numpy as np

from concourse import bacc, bass, tile
from concourse.bass_utils import run_bass_kernel_spmd

mybir = bass.mybir
F32 = mybir.dt.float32
BF16 = mybir.dt.bfloat16
I8 = mybir.dt.int8

B, F = 16384, 2048
N_CORES = 8
ROWS = B // N_CORES  # 2048 rows per core
P = 128
N_FT = F // P  # 16 feature tiles
RC = 512  # rows per PSUM bank (512 f32 = one bank)
N_RC = ROWS // RC  # 4

N_WARMUP = int(os.environ.get("KWARMUP", "48"))

# Chunk schedule: (f-tiles, dtype, ring), consumed by the PE in order.
# Chunk 0 is bf16 and carries w in 16 extra columns.
CHUNKS = [
    ((0,), "bf16", "sync"),
    ((1, 2), "int8", "scalar"),
    ((3, 4), "int8", "sync"),
    ((5, 6), "int8", "scalar"),
    ((7, 8), "int8", "sync"),
    ((9, 10), "int8", "scalar"),
    ((11, 12), "bf16", "sync"),
    ((13, 14, 15), "bf16", "scalar"),
]
BF16_FTS = sorted(ft for fts, dt, _ in CHUNKS if dt == "bf16" for ft in fts)
I8_FTS = sorted(ft for fts, dt, _ in CHUNKS if dt == "int8" for ft in fts)

_NC = None
LAST_RESULT = None  # BassKernelResults of the most recent run (for test.py)


def _build_nc():
    nc = bacc.Bacc("TRN2", target_bir_lowering=False, debug=False)
    # Chunk 0: f-tile 0 plus 16 columns of w. Other bf16 tiles in xt16,
    # int8 tiles in xt8 — each packed in f-tile order.
    c0 = nc.dram_tensor("c0", [P, ROWS + N_FT], BF16, kind="ExternalInput")
    xt8 = nc.dram_tensor("xt8", [len(I8_FTS) * P, ROWS], I8, kind="ExternalInput")
    xt16 = nc.dram_tensor(
        "xt16", [(len(BF16_FTS) - 1) * P, ROWS], BF16, kind="ExternalInput"
    )
    out = nc.dram_tensor("out", [1, ROWS], F32, kind="ExternalOutput")
    i8_pos = {ft: i for i, ft in enumerate(I8_FTS)}
    bf_pos = {ft: i for i, ft in enumerate(BF16_FTS[1:])}

    with tile.TileContext(nc) as tc:
        with (
            tc.tile_pool(name="xi", bufs=1) as xi,
            tc.tile_pool(name="xb", bufs=1) as xb,
            tc.tile_pool(name="wp", bufs=1) as wp,
            tc.psum_pool(name="pp", bufs=1) as pp,
        ):
            # PE HAM warmup: garbage matmuls (no data deps) keep the PE busy
            # from t=0 so the clock gate is open when real tiles arrive.
            warm_t = wp.tile([P, P], BF16)
            psums = [
                pp.tile([1, RC], F32, name=f"psum{rc}") for rc in range(N_RC)
            ]
            if N_WARMUP:
                warm_ps = pp.tile([1, P], F32)
                nc.gpsimd.memset(warm_t[:], 0)
                for _ in range(N_WARMUP):
                    nc.tensor.matmul(
                        warm_ps[:, :], lhsT=warm_t[:, 0:1], rhs=warm_t[:],
                        start=True, stop=True,
                    )

            out_t = wp.tile([1, ROWS], F32)
            c0_t = None
            tiles = {}
            for ci, (fts, dt, ring) in enumerate(CHUNKS):
                size = len(fts)
                eng = nc.sync if ring == "sync" else nc.scalar
                if ci == 0:
                    c0_t = wp.tile([P, ROWS + N_FT], BF16)
                    eng.dma_start(c0_t[:], c0.ap())
                    tiles[0] = (c0_t, None)
                    continue
                if dt == "int8":
                    t0 = i8_pos[fts[0]]
                    tl = xi.tile([P, size, ROWS], I8, name=f"xi{ci}")
                    src = xt8.ap()[t0 * P : (t0 + size) * P, :]
                else:
                    t0 = bf_pos[fts[0]]
                    tl = xb.tile([P, size, ROWS], BF16, name=f"xb{ci}")
                    src = xt16.ap()[t0 * P : (t0 + size) * P, :]
                eng.dma_start(tl[:], src.rearrange("(g p) r -> p g r", p=P))
                if dt == "int8":
                    xc = xb.tile([P, size, ROWS], BF16, name=f"xc{ci}")
                    nc.vector.tensor_copy(out=xc[:], in_=tl[:])
                    tl = xc
                for g, ft in enumerate(fts):
                    tiles[ft] = (tl, g)

            for ft in range(N_FT):
                tl, g = tiles[ft]
                for rc in range(N_RC):
                    rhs = (
                        tl[:, rc * RC : (rc + 1) * RC]
                        if g is None
                        else tl[:, g, rc * RC : (rc + 1) * RC]
                    )
                    nc.tensor.matmul(
                        psums[rc][:, :],
                        lhsT=c0_t[:, ROWS + ft : ROWS + ft + 1],
                        rhs=rhs,
                        start=(ft == 0),
                        stop=(ft == N_FT - 1),
                    )

            # Per-bank evacuation; each copy only waits for its own bank's
            # last matmul. Two half-outputs, one per ring.
            for rc in range(N_RC):
                dst = out_t[:, rc * RC : (rc + 1) * RC]
                if rc % 2 == 0:
                    nc.scalar.copy(out=dst, in_=psums[rc][:, :])
                else:
                    nc.vector.tensor_copy(out=dst, in_=psums[rc][:, :])
            half = ROWS // 2
            nc.scalar.dma_start(out.ap()[:, :half], out_t[:, :half])
            nc.sync.dma_start(out.ap()[:, half:], out_t[:, half:])
    nc.finalize()
    return nc


def kernel(x, group_idx, gc_w, diag_w, fc_w):
    global _NC, LAST_RESULT
    x = np.ascontiguousarray(np.asarray(x, dtype=np.float32))
    gi = np.asarray(group_idx).astype(np.int64)
    gc_w = np.asarray(gc_w, dtype=np.float32)
    diag_w = np.asarray(diag_w, dtype=np.float32).reshape(-1)
    fc_w = np.asarray(fc_w, dtype=np.float32).reshape(-1, 1)

    # Fold everything linear into one combined weight vector (exact).
    coef = gc_w * diag_w[:, None] * fc_w  # [256, 8]
    w = np.zeros(F, dtype=np.float32)
    np.add.at(w, gi.ravel(), coef.ravel().astype(np.float32))
    # stationary layout: wst[p, t] = w[t*128 + p]
    wst = np.ascontiguousarray(w.reshape(N_FT, P).T).astype(ml_dtypes.bfloat16)

    # Per-row scales; bf16 tiles are sent pre-scaled by 1/d_r too, so one
    # host-side out *= d_r fixes everything.
    d = np.maximum(np.abs(x).max(axis=1), 1e-30) / 127.0  # [B]
    xs = x / d[:, None]
    xq = np.rint(xs).astype(np.int8)
    xb16 = xs.astype(ml_dtypes.bfloat16)

    i8_rows = np.array([list(range(ft * P, (ft + 1) * P)) for ft in I8_FTS]
                       ).reshape(-1)
    bf_rows = np.array(
        [list(range(ft * P, (ft + 1) * P)) for ft in BF16_FTS[1:]]
    ).reshape(-1)
    in_maps = []
    for i in range(N_CORES):
        sl = slice(i * ROWS, (i + 1) * ROWS)
        xT = xb16[sl].T
        c0b = np.concatenate([xT[:P], wst], axis=1)  # [128, ROWS+16]
        in_maps.append({
            "c0": np.ascontiguousarray(c0b),
            "xt8": np.ascontiguousarray(xq[sl].T[i8_rows]),
            "xt16": np.ascontiguousarray(xT[bf_rows]),
        })

    if _NC is None:
        _NC = _build_nc()

    trace = bool(int(os.environ.get("TRN_KERNEL_TRACE", "0")))
    LAST_RESULT = run_bass_kernel_spmd(
        _NC, in_maps, list(range(N_CORES)), trace=trace
    )
    outs = [
        LAST_RESULT.results[i]["out"].reshape(ROWS).astype(np.float32)
        for i in range(N_CORES)
    ]
    full = np.concatenate(outs) * d
    return full.reshape(B, 1).astype(np.float32)
